# revision 1
# baseline (speedup 1.0000x reference)
"""Multi-head attention (B=2, S=2048, d_model=768, H=12) on 8 TRN2 NeuronCores.

Sharding: 2-way data parallel over batch x 4-way tensor parallel over heads
(3 heads / 192-wide d_model slice per core). Host compacts masked keys away
(gather of unmasked key/value rows), pads to a 128 multiple, zero-fills pad
keys; softmax needs no mask handling on device (pad keys get V=0 and 0s in
the denominator ones-block). Host pre-rearranges projection weights into the
on-chip [128, kt, m] layout so weight DMAs are single contiguous transfers;
xq/xv load as one batched strided DMA each (xq chunk-0 first).

Per core, a software-pipelined flat loop over (chunk c of 512 q, key tile t):

  scores^T[k,q] h0/h1 via disjoint PE row groups (2x concurrency) into one
  3-bank PSUM tile [128,1536] (h0|h1|h2), consumed by two exp instructions
  (ACT: [0:1024] then [1024:1536]) so banks release early; h2 single matmul
  (two concurrent same-bank matmuls deadlock the device). The NEXT tile's
  score matmuls are emitted directly after this tile's exp and PV trails one
  iteration behind, so the in-order PE always has score work queued when exp
  frees a bank.

  V blocks of 128 cols/head, all [valid-ones 64 | V 64]: PV lands a
  64-row-replicated denominator on partitions 0:64 and ctx on 64:128 of one
  3-bank ctx PSUM tile. Normalization per head section: fast reciprocal
  straight from PSUM (base-0 partitions only: the custom DVE op corrupts
  memory at partition offset 64) -> psum*recip multiply to bf16. The first
  PV of each chunk trails one extra iteration so the previous chunk's norm
  clears the ctx banks before the in-order PE reaches the start=True PV.

  Output projection runs transposed (outT[dm,q] = Wo_g^T @ ctx, 2
  accumulating matmuls per 128-row dm tile, one PSUM bank), processed in
  adjacent-tile pairs so the two 64-contraction WO2 matmuls run in disjoint
  PE row groups into different PSUM banks (concurrent); bf16 partial
  outputs are transposed and summed on host with bo. The pipeline starts
  right after the first K key-chunk projects; the remaining K chunks, Q
  projections for chunk c+1, V projection tiles, and the O-projection of
  chunk c-1 are threaded between attention iterations to keep the PE
  HAM-warm (scores for tile t only need keys up to t*128, which stay ahead).
  Projection evictions carry the bias via ACT Identity / DVE tensor_scalar;
  a tiny phase-A exp preloads the ACT exp table set off the critical path.
"""

import math

import numpy as np

B = 2
S = 2048
DM = 768
H = 12
DH = 64
G = 4              # head-group (tensor-parallel) degree
HPG = H // G       # heads per core
DQ = HPG * DH      # 192 d_model slice per core
NCORES = 8
P = 128
QC = 512           # query chunk
NQC = S // QC

_prog_cache = {}


def _build_nc(KP):
    import concourse.mybir as mybir
    import concourse.tile as tile
    from concourse import bacc

    F32 = mybir.dt.float32
    BF = mybir.dt.bfloat16
    AFT = mybir.ActivationFunctionType

    T = KP // P            # key tiles
    NKT = DM // P          # 6 contraction tiles for projections
    KCH = [(o, min(512, KP - o)) for o in range(0, KP, 512)]

    nc = bacc.Bacc(None, target_bir_lowering=False)
    xqT = nc.declare_dram_parameter("xqT", [DM, S], BF, isOutput=False)
    xkT = nc.declare_dram_parameter("xkT", [DM, KP], BF, isOutput=False)
    xvT = nc.declare_dram_parameter("xvT", [DM, KP], BF, isOutput=False)
    # weights pre-rearranged on host to [128, NKT*DQ] (p-major)
    wq = nc.declare_dram_parameter("wq", [P, NKT * DQ], BF, isOutput=False)
    wk = nc.declare_dram_parameter("wk", [P, NKT * DQ], BF, isOutput=False)
    wv = nc.declare_dram_parameter("wv", [P, NKT * DQ], BF, isOutput=False)
    wo = nc.declare_dram_parameter("wo", [DQ, DM], BF, isOutput=False)
    bq = nc.declare_dram_parameter("bq", [DQ, 1], F32, isOutput=False)
    bk = nc.declare_dram_parameter("bk", [DQ, 1], F32, isOutput=False)
    bv = nc.declare_dram_parameter("bv", [1, DQ], F32, isOutput=False)
    vm = nc.declare_dram_parameter("vm", [P, T], F32, isOutput=False)
    out = nc.declare_dram_parameter("out", [DM, S], BF, isOutput=True)

    with tile.TileContext(nc) as tc:
        with (
            tc.tile_pool(name="persist", bufs=1) as persist,
            tc.tile_pool(name="xin", bufs=6) as xin,
            tc.tile_pool(name="es", bufs=4) as espool,
            tc.tile_pool(name="rc", bufs=3) as rcpool,
            tc.tile_pool(name="osb", bufs=4) as osb,
            tc.tile_pool(name="ps_sp", bufs=1, space="PSUM") as ps_sp,
            tc.tile_pool(name="ps_ctx", bufs=1, space="PSUM") as ps_ctx,
            tc.tile_pool(name="ps_w", bufs=2, space="PSUM") as ps_w,
        ):
            # ---- constants / weights (K path first) ----
            BQ0 = persist.tile([P, 1], F32, tag="BQ0")
            BQ1 = persist.tile([DH, 1], F32, tag="BQ1")
            BK0 = persist.tile([P, 1], F32, tag="BK0")
            BK1 = persist.tile([DH, 1], F32, tag="BK1")
            BV = persist.tile([P, DQ], F32, tag="BV")
            VM = persist.tile([P, T], F32, tag="VM")

            WK = persist.tile([P, NKT, DQ], BF, tag="WK")
            nc.sync.dma_start(out=WK, in_=wk[:, :].rearrange("p (kt m) -> p kt m", m=DQ))
            XK = []
            for kt in range(NKT):
                xt = xin.tile([P, KP], BF, tag="xk", name=f"xk{kt}")
                nc.sync.dma_start(out=xt, in_=xkT[kt * P:(kt + 1) * P, :])
                XK.append(xt)
            WQ = persist.tile([P, NKT, DQ], BF, tag="WQ")
            nc.sync.dma_start(out=WQ, in_=wq[:, :].rearrange("p (kt m) -> p kt m", m=DQ))
            XQA = persist.tile([P, NKT, S], BF, tag="xqa")
            nc.sync.dma_start(
                out=XQA[:, :, 0:QC],
                in_=xqT[:, 0:QC].rearrange("(kt p) q -> p kt q", p=P),
            )
            nc.sync.dma_start(out=BK0, in_=bk[0:P, :])
            nc.sync.dma_start(out=BK1, in_=bk[P:DQ, :])
            nc.sync.dma_start(out=BQ0, in_=bq[0:P, :])
            nc.sync.dma_start(out=BQ1, in_=bq[P:DQ, :])
            nc.sync.dma_start(out=BV, in_=bv[:, :].to_broadcast([P, DQ]))
            nc.sync.dma_start(out=VM, in_=vm[:, :])
            WV = persist.tile([P, NKT, DQ], BF, tag="WV")
            nc.sync.dma_start(out=WV, in_=wv[:, :].rearrange("p (kt m) -> p kt m", m=DQ))
            XVA = persist.tile([P, NKT, KP], BF, tag="xva")
            nc.sync.dma_start(
                out=XVA, in_=xvT[:, :].rearrange("(kt p) k -> p kt k", p=P)
            )
            nc.sync.dma_start(
                out=XQA[:, :, QC:S],
                in_=xqT[:, QC:S].rearrange("(kt p) q -> p kt q", p=P),
            )
            WO0 = persist.tile([P, DM], BF, tag="WO0")   # wo rows 0:128 (h0,h1)
            WO2 = persist.tile([DH, DM], BF, tag="WO2")  # wo rows 128:192 (h2)
            WO2x = persist.tile([P, DM], BF, tag="WO2x")  # dup on rows 64:128
            nc.sync.dma_start(out=WO0, in_=wo[0:P, :])
            nc.sync.dma_start(out=WO2, in_=wo[P:DQ, :])
            nc.sync.dma_start(out=WO2x[DH:P, :], in_=wo[P:DQ, :])

            # ---- persistent activations ----
            QT0 = persist.tile([P, S], BF, tag="QT0")    # heads 0,1
            QT1 = persist.tile([DH, S], BF, tag="QT1")   # head 2
            KT0 = persist.tile([P, KP], BF, tag="KT0")
            KT1 = persist.tile([DH, KP], BF, tag="KT1")  # head 2
            # V blocks of 128 cols per head (see module docstring)
            VP = persist.tile([P, T, HPG * P], BF, tag="VP")
            CTX01 = persist.tile([P, S], BF, tag="CTX01")  # h0 rows 0:64, h1 64:128
            CTX2 = persist.tile([DH, S], BF, tag="CTX2")   # h2
            CTX2x = persist.tile([P, S], BF, tag="CTX2x")  # h2 dup on rows 64:128

            ONES = persist.tile([P, HPG * DH], BF, tag="ONES")
            nc.vector.memset(ONES, 1.0)
            # pre-load the ACT exp table set during the idle projection phase
            WRM = persist.tile([1, 2], F32, tag="WRM")
            nc.vector.memset(WRM, 0.0)
            nc.scalar.activation(WRM[:, 1:2], WRM[:, 0:1], AFT.Exp)

            # ---- K projection (per key chunk; later chunks threaded into
            # the early attention iterations, DVE eviction) ----
            def kproj_chunk(kc, evict_on_act):
                c0, cw = KCH[kc]
                for m, (bias, mw) in enumerate([(BK0, P), (BK1, DH)]):
                    ps = ps_w.tile([P, 512], F32, tag="psw", name=f"kps{c0}_{m}")
                    for kt in range(NKT):
                        nc.tensor.matmul(
                            ps[0:mw, 0:cw],
                            lhsT=WK[:, kt, m * P:m * P + mw],
                            rhs=XK[kt][:, c0:c0 + cw],
                            start=(kt == 0),
                            stop=(kt == NKT - 1),
                        )
                    dst = KT0 if m == 0 else KT1
                    if evict_on_act:
                        nc.scalar.activation(
                            dst[0:mw, c0:c0 + cw], ps[0:mw, 0:cw],
                            AFT.Identity, bias=bias,
                        )
                    else:
                        nc.vector.tensor_scalar_add(
                            dst[0:mw, c0:c0 + cw], ps[0:mw, 0:cw], bias
                        )

            kproj_chunk(0, True)

            _qps = {}

            def qproj_part(c, m, half, evict_on_act=False):
                c0 = c * QC
                bias, mw = (BQ0, P) if m == 0 else (BQ1, DH)
                if half == 0:
                    _qps[(c, m)] = ps_w.tile(
                        [P, 512], F32, tag="psw", name=f"qps{c}_{m}"
                    )
                ps = _qps[(c, m)]
                kts = range(0, NKT // 2) if half == 0 else range(NKT // 2, NKT)
                for kt in kts:
                    nc.tensor.matmul(
                        ps[0:mw, :],
                        lhsT=WQ[:, kt, m * P:m * P + mw],
                        rhs=XQA[:, kt, c0:c0 + QC],
                        start=(kt == 0),
                        stop=(kt == NKT - 1),
                    )
                if half == 0:
                    return
                dst = QT0 if m == 0 else QT1
                if evict_on_act:
                    nc.scalar.activation(
                        dst[0:mw, c0:c0 + QC], ps[0:mw, :], AFT.Identity, bias=bias
                    )
                else:
                    nc.vector.tensor_scalar_add(
                        dst[0:mw, c0:c0 + QC], ps[0:mw, :], bias
                    )
                del _qps[(c, m)]

            def qproj_half(c, m, evict_on_act=False):
                qproj_part(c, m, 0, evict_on_act)
                qproj_part(c, m, 1, evict_on_act)

            def vproj(t):
                ps = ps_w.tile([P, 512], F32, tag="psw", name=f"vps{t}")
                for kt in range(NKT):
                    nc.tensor.matmul(
                        ps[:, 0:DQ],
                        lhsT=XVA[:, kt, t * P:(t + 1) * P],
                        rhs=WV[:, kt, :],
                        start=(kt == 0),
                        stop=(kt == NKT - 1),
                    )
                # all blocks [valid-ones 0:64 | V 64:128]: PV rows 0:64 = den
                # (partition base 0 for the fused psum reciprocal), 64:128 ctx
                vview = VP[:, t, :].rearrange("p (h c) -> p h c", c=P)
                nc.vector.tensor_add(
                    vview[:, :, DH:P],
                    ps[:, 0:DQ].rearrange("p (h d) -> p h d", d=DH),
                    BV[:, :].rearrange("p (h d) -> p h d", d=DH),
                )
                nc.vector.tensor_scalar_mul(
                    vview[:, :, DH:P], vview[:, :, DH:P], VM[:, t:t + 1]
                )
                nc.vector.tensor_scalar_mul(
                    vview[:, :, 0:DH],
                    ONES[:, :].rearrange("p (h d) -> p h d", d=DH),
                    VM[:, t:t + 1],
                )

            qproj_half(0, 0, evict_on_act=True)
            qproj_half(0, 1, evict_on_act=True)

            # ---- attention, software-pipelined emission ----
            sp = ps_sp.tile([P, 1536], F32, tag="sp")      # [h0 512 | h1 512 | h2 512]
            ctx = ps_ctx.tile([P, 1536], F32, tag="ctx")   # [h0 | h1 | h2]
            SCL = 1.0 / math.sqrt(DH)

            def scores01(c, t):
                c0 = c * QC
                tsl = slice(t * P, (t + 1) * P)
                nc.tensor.matmul(
                    sp[:, 0:512],
                    lhsT=KT0[0:DH, tsl], rhs=QT0[0:DH, c0:c0 + QC],
                    start=True, stop=True,
                )
                nc.tensor.matmul(
                    sp[:, 512:1024],
                    lhsT=KT0[DH:P, tsl], rhs=QT0[DH:P, c0:c0 + QC],
                    start=True, stop=True,
                )

            def scoresh2(c, t):
                c0 = c * QC
                tsl = slice(t * P, (t + 1) * P)
                nc.tensor.matmul(
                    sp[:, 1024:1536],
                    lhsT=KT1[0:DH, tsl], rhs=QT1[0:DH, c0:c0 + QC],
                    start=True, stop=True,
                )

            def oproj_one(c, mo):
                # outT[mo*128:(mo+1)*128, chunk c] = WO0[:,mo].T@CTX01 + WO2[:,mo].T@CTX2
                c0 = c * QC
                po = ps_w.tile([P, 512], F32, tag="psw", name=f"po{c}_{mo}")
                osl = slice(mo * P, (mo + 1) * P)
                nc.tensor.matmul(
                    po, lhsT=WO0[:, osl], rhs=CTX01[:, c0:c0 + QC],
                    start=True, stop=False,
                )
                nc.tensor.matmul(
                    po, lhsT=WO2[:, osl], rhs=CTX2[0:DH, c0:c0 + QC],
                    start=False, stop=True,
                )
                po_sb = osb.tile([P, 512], BF, tag="posb", name=f"posb{c}_{mo}")
                nc.vector.tensor_copy(po_sb, po)
                nc.sync.dma_start(out=out[osl, c0:c0 + QC], in_=po_sb)

            def oproj_pair(c, mo):
                # tiles mo, mo+1: the two 64-contraction WO2 matmuls run in
                # disjoint PE row groups into different PSUM banks
                c0 = c * QC
                po_a = ps_w.tile([P, 512], F32, tag="psw", name=f"pa{c}_{mo}")
                po_b = ps_w.tile([P, 512], F32, tag="psw", name=f"pb{c}_{mo}")
                sa = slice(mo * P, (mo + 1) * P)
                sb = slice((mo + 1) * P, (mo + 2) * P)
                nc.tensor.matmul(
                    po_a, lhsT=WO0[:, sa], rhs=CTX01[:, c0:c0 + QC],
                    start=True, stop=False,
                )
                nc.tensor.matmul(
                    po_b, lhsT=WO0[:, sb], rhs=CTX01[:, c0:c0 + QC],
                    start=True, stop=False,
                )
                nc.tensor.matmul(
                    po_a, lhsT=WO2[:, sa], rhs=CTX2[0:DH, c0:c0 + QC],
                    start=False, stop=True,
                )
                nc.tensor.matmul(
                    po_b, lhsT=WO2x[DH:P, sb], rhs=CTX2x[DH:P, c0:c0 + QC],
                    start=False, stop=True,
                )
                for po_x, sx in ((po_a, sa), (po_b, sb)):
                    po_sb = osb.tile(
                        [P, 512], BF, tag="posb", name=f"ps{c}_{sx.start}"
                    )
                    nc.vector.tensor_copy(po_sb, po_x)
                    nc.sync.dma_start(out=out[sx, c0:c0 + QC], in_=po_sb)

            def pv(c, t):
                for h in range(HPG):
                    nc.tensor.matmul(
                        ctx[:, h * 512:h * 512 + 512],
                        lhsT=VP[:, t, h * P:(h + 1) * P],
                        rhs=es_hist[c * T + t][:, h * 512:(h + 1) * 512],
                        start=(t == 0), stop=(t == T - 1),
                    )

            def norm(c):
                # den rows 0:64 of every ctx section (partition base 0):
                # one fused reciprocal psum->sbuf, then per-head multiplies.
                c0 = c * QC
                rc = rcpool.tile([DH, 1536], F32, tag="rc", name=f"rc{c}")
                for h, (dst, dr) in enumerate([
                    (CTX01, slice(0, DH)),
                    (CTX01, slice(DH, P)),
                    (CTX2, slice(0, DH)),
                ]):
                    cs = slice(h * 512, (h + 1) * 512)
                    nc.vector.reciprocal_approx_fast(rc[:, cs], ctx[0:DH, cs])
                    nc.vector.tensor_mul(
                        dst[dr, c0:c0 + QC], ctx[DH:P, cs], rc[:, cs]
                    )
                nc.vector.tensor_mul(
                    CTX2x[DH:P, c0:c0 + QC], ctx[DH:P, 1024:1536],
                    rc[:, 1024:1536],
                )

            # Flat pipeline over i = c*T + t: exp(i) | scores(i+1) | PV(i-1).
            # PV trails one iteration so the PE's in-order queue always has
            # the next scores directly behind the current exp, and the norm
            # chain gets a full iteration of cushion before the next chunk's
            # first PV (ctx WAR) can stall the PE.
            NI = NQC * T
            LAG = 1
            es_hist = {}
            scores01(0, 0)
            scoresh2(0, 0)
            for i in range(NI + LAG):
                c, t = divmod(min(i, NI - 1), T)
                if i < NI:
                    es = espool.tile([P, 1536], BF, tag="es", name=f"es{c}_{t}")
                    es_hist[c * T + t] = es
                    nc.scalar.activation(
                        es[:, 0:1024], sp[:, 0:1024], AFT.Exp, bias=0.0, scale=SCL
                    )
                    nc.scalar.activation(
                        es[:, 1024:1536], sp[:, 1024:1536], AFT.Exp,
                        bias=0.0, scale=SCL,
                    )
                    nxt_ok = i + 1 < NI and ((i + 1) % T != 0 or T >= 3)
                    if nxt_ok:
                        scores01(*divmod(i + 1, T))
                if i >= LAG:
                    pc, pt = divmod(i - LAG, T)
                    # first PV of a chunk trails one extra iteration so the
                    # previous chunk's norm clears the ctx banks in time
                    if pt == 0 and T >= 3:
                        pass
                    elif pt == 1 and T >= 3:
                        pv(pc, 0)
                        es_hist.pop(pc * T)
                        pv(pc, 1)
                        es_hist.pop(pc * T + 1)
                    else:
                        pv(pc, pt)
                        es_hist.pop(pc * T + pt)
                if i < NI and nxt_ok:
                    scoresh2(*divmod(i + 1, T))
                if i >= LAG and (i - LAG) % T == T - 1:
                    norm((i - LAG) // T)
                # threaded non-attention work, keyed by the exp index (c, t)
                if c == 0 and i < NI and t + 1 < len(KCH):
                    kproj_chunk(t + 1, False)
                if c == 0 and i < NI:
                    if t == 0:
                        vproj(0)
                        if T > 1:
                            vproj(1)
                    elif t + 1 < T:
                        vproj(t + 1)
                if c > 0 and i < NI and T >= 7 and t in (1, 3, 5):
                    oproj_pair(c - 1, t - 1)
                if c > 0 and i < NI and T < 7 and 1 <= t <= min(DM // P, T - 1):
                    oproj_one(c - 1, t - 1)
                if c + 1 < NQC and T >= 3 and i < NI:
                    if t == T - 3:
                        qproj_half(c + 1, 0)
                    elif t == T - 2:
                        qproj_half(c + 1, 1)
                if c + 1 < NQC and T < 3 and i < NI and t == T - 1:
                    qproj_half(c + 1, 0)
                    qproj_half(c + 1, 1)
                    scores01(c + 1, 0)
                    scoresh2(c + 1, 0)
                # leftover O-proj tiles when T is too small to thread them all
                if c > 0 and t == T - 1 and i < NI and T < 7:
                    for mo in range(min(DM // P, T - 1), DM // P):
                        oproj_one(c - 1, mo)
            if T >= 7:
                for mo in (0, 2, 4):
                    oproj_pair(NQC - 1, mo)
            else:
                for mo in range(DM // P):
                    oproj_one(NQC - 1, mo)
    nc.compile()
    return nc


def _get_prog(KP):
    if KP not in _prog_cache:
        _prog_cache[KP] = _build_nc(KP)
    return _prog_cache[KP]


def _rearrange_w(Wslice, BF):
    # [768, 192] -> [128, 6*192] (p-major kt blocks), contiguous for DMA
    return np.ascontiguousarray(
        Wslice.reshape(DM // P, P, DQ).transpose(1, 0, 2).reshape(P, -1)
    ).astype(BF)


def _run(inputs, trace=False):
    import ml_dtypes
    from concourse.bass_utils import run_bass_kernel_spmd

    BF = ml_dtypes.bfloat16

    query = np.asarray(inputs["query"], dtype=np.float32)
    key = np.asarray(inputs["key"], dtype=np.float32)
    value = np.asarray(inputs["value"], dtype=np.float32)
    mask = np.asarray(inputs["mask"])
    Wq = np.asarray(inputs["Wq"], dtype=np.float32)
    bq = np.asarray(inputs["bq"], dtype=np.float32)
    Wk = np.asarray(inputs["Wk"], dtype=np.float32)
    bk = np.asarray(inputs["bk"], dtype=np.float32)
    Wv = np.asarray(inputs["Wv"], dtype=np.float32)
    bv = np.asarray(inputs["bv"], dtype=np.float32)
    Wo = np.asarray(inputs["Wo"], dtype=np.float32)
    bo = np.asarray(inputs["bo"], dtype=np.float32)

    idx = [np.nonzero(mask[b, 0, 0] != 0)[0] for b in range(B)]
    keff = [len(i) for i in idx]
    KP = max(P, ((max(keff) + P - 1) // P) * P)
    T = KP // P

    nc = _get_prog(KP)

    per_batch = {}
    for b in range(B):
        xqT = np.ascontiguousarray(query[b].T).astype(BF)
        xkT = np.zeros((DM, KP), dtype=BF)
        xkT[:, :keff[b]] = key[b][idx[b]].T.astype(BF)
        xvT = np.zeros((DM, KP), dtype=BF)
        xvT[:, :keff[b]] = value[b][idx[b]].T.astype(BF)
        vmf = np.zeros((KP,), dtype=np.float32)
        vmf[:keff[b]] = 1.0
        vm2 = np.ascontiguousarray(vmf.reshape(T, P).T)  # [128, T]
        per_batch[b] = (xqT, xkT, xvT, vm2)

    in_maps = []
    for core in range(NCORES):
        b, g = core // G, core % G
        xqT, xkT, xvT, vm2 = per_batch[b]
        sl = slice(g * DQ, (g + 1) * DQ)
        in_maps.append({
            "xqT": xqT,
            "xkT": xkT,
            "xvT": xvT,
            "wq": _rearrange_w(Wq[:, sl], BF),
            "wk": _rearrange_w(Wk[:, sl], BF),
            "wv": _rearrange_w(Wv[:, sl], BF),
            "wo": np.ascontiguousarray(Wo[sl, :]).astype(BF),
            "bq": np.ascontiguousarray(bq[sl].reshape(DQ, 1)),
            "bk": np.ascontiguousarray(bk[sl].reshape(DQ, 1)),
            "bv": np.ascontiguousarray(bv[sl].reshape(1, DQ)),
            "vm": vm2,
        })

    res = run_bass_kernel_spmd(nc, in_maps, list(range(NCORES)), trace=trace)

    outp = np.zeros((B, S, DM), dtype=np.float32)
    for core in range(NCORES):
        outp[core // G] += np.asarray(res.results[core]["out"], dtype=np.float32).T
    outp += bo.reshape(1, 1, DM)
    return outp, res


def kernel(**inputs) -> np.ndarray:
    out, _ = _run(inputs, trace=False)
    return out


if __name__ == "__main__":
    nc = _build_nc(1152)
    print("build OK")



# revision 4
# speedup vs baseline: 1.1098x; 1.1098x over previous
"""Multi-head attention (B=2, S=2048, d_model=768, H=12) on 8 TRN2 NeuronCores.

Sharding: 2-way data parallel over batch x 4-way tensor parallel over heads
(3 heads / 192-wide d_model slice per core). Host compacts masked keys away
(gather of unmasked key/value rows), pads to a 128 multiple, zero-fills pad
keys; softmax needs no mask handling on device (pad keys get V=0 and 0s in
the denominator ones-block). Host pre-rearranges projection weights into the
on-chip [128, kt, m] layout so weight DMAs are single contiguous transfers;
xq/xv load as one batched strided DMA each (xq chunk-0 first).

Per core, a software-pipelined flat loop over (chunk c of 512 q, key tile t):

  scores^T[k,q] h0/h1 via disjoint PE row groups (2x concurrency) into one
  3-bank PSUM tile [128,1536] (h0|h1|h2), consumed by two exp instructions
  (ACT: [0:1024] then [1024:1536]) so banks release early; h2 single matmul
  (two concurrent same-bank matmuls deadlock the device). The NEXT tile's
  score matmuls are emitted directly after this tile's exp and PV trails one
  iteration behind, so the in-order PE always has score work queued when exp
  frees a bank.

  V blocks of 128 cols/head, all [valid-ones 64 | V 64]: PV lands a
  64-row-replicated denominator on partitions 0:64 and ctx on 64:128 of one
  3-bank ctx PSUM tile. Normalization per head section: fast reciprocal
  straight from PSUM (base-0 partitions only: the custom DVE op corrupts
  memory at partition offset 64) -> psum*recip multiply to bf16. The first
  PV of each chunk trails one extra iteration so the previous chunk's norm
  clears the ctx banks before the in-order PE reaches the start=True PV.

  Output projection runs transposed (outT[dm,q] = Wo_g^T @ ctx, 2
  accumulating matmuls per 128-row dm tile, one PSUM bank), processed in
  adjacent-tile pairs so the two 64-contraction WO2 matmuls run in disjoint
  PE row groups into different PSUM banks (concurrent); bf16 partial
  outputs are transposed and summed on host with bo. The pipeline starts
  right after the first K key-chunk projects; the remaining K chunks, Q
  projections for chunk c+1, V projection tiles, and the O-projection of
  chunk c-1 are threaded between attention iterations to keep the PE
  HAM-warm (scores for tile t only need keys up to t*128, which stay ahead).
  Projection evictions carry the bias via ACT Identity / DVE tensor_scalar;
  a tiny phase-A exp preloads the ACT exp table set off the critical path.
"""

import math

import numpy as np

B = 2
S = 2048
DM = 768
H = 12
DH = 64
G = 4              # head-group (tensor-parallel) degree
HPG = H // G       # heads per core
DQ = HPG * DH      # 192 d_model slice per core
NCORES = 8
P = 128
QC = 512           # query chunk
NQC = S // QC

_prog_cache = {}


def _build_nc(KP):
    import concourse.mybir as mybir
    import concourse.tile as tile
    from concourse import bacc

    F32 = mybir.dt.float32
    BF = mybir.dt.bfloat16
    AFT = mybir.ActivationFunctionType

    T = KP // P            # key tiles
    NKT = DM // P          # 6 contraction tiles for projections
    KCH = [(o, min(512, KP - o)) for o in range(0, KP, 512)]

    nc = bacc.Bacc(None, target_bir_lowering=False)
    xqT = nc.declare_dram_parameter("xqT", [DM, S], BF, isOutput=False)
    xkT = nc.declare_dram_parameter("xkT", [DM, KP], BF, isOutput=False)
    xvT = nc.declare_dram_parameter("xvT", [DM, KP], BF, isOutput=False)
    # weights pre-rearranged on host to [128, NKT*DQ] (p-major)
    wq = nc.declare_dram_parameter("wq", [P, NKT * DQ], BF, isOutput=False)
    wk = nc.declare_dram_parameter("wk", [P, NKT * DQ], BF, isOutput=False)
    wv = nc.declare_dram_parameter("wv", [P, NKT * DQ], BF, isOutput=False)
    wo = nc.declare_dram_parameter("wo", [DQ, DM], BF, isOutput=False)
    bq = nc.declare_dram_parameter("bq", [DQ, 1], F32, isOutput=False)
    bk = nc.declare_dram_parameter("bk", [DQ, 1], F32, isOutput=False)
    bv = nc.declare_dram_parameter("bv", [1, DQ], F32, isOutput=False)
    vm = nc.declare_dram_parameter("vm", [P, T], F32, isOutput=False)
    out = nc.declare_dram_parameter("out", [DM, S], BF, isOutput=True)

    with tile.TileContext(nc) as tc:
        with (
            tc.tile_pool(name="persist", bufs=1) as persist,
            tc.tile_pool(name="xin", bufs=6) as xin,
            tc.tile_pool(name="es", bufs=4) as espool,
            tc.tile_pool(name="rc", bufs=3) as rcpool,
            tc.tile_pool(name="osb", bufs=4) as osb,
            tc.tile_pool(name="ps_sp", bufs=1, space="PSUM") as ps_sp,
            tc.tile_pool(name="ps_ctx", bufs=1, space="PSUM") as ps_ctx,
            tc.tile_pool(name="ps_w", bufs=2, space="PSUM") as ps_w,
        ):
            # ---- constants / weights (K path first) ----
            BQ0 = persist.tile([P, 1], F32, tag="BQ0")
            BQ1 = persist.tile([DH, 1], F32, tag="BQ1")
            BK0 = persist.tile([P, 1], F32, tag="BK0")
            BK1 = persist.tile([DH, 1], F32, tag="BK1")
            BV = persist.tile([P, DQ], F32, tag="BV")
            VM = persist.tile([P, T], F32, tag="VM")

            WK = persist.tile([P, NKT, DQ], BF, tag="WK")
            nc.sync.dma_start(out=WK, in_=wk[:, :].rearrange("p (kt m) -> p kt m", m=DQ))
            XK = []
            for kt in range(NKT):
                xt = xin.tile([P, KP], BF, tag="xk", name=f"xk{kt}")
                nc.sync.dma_start(out=xt, in_=xkT[kt * P:(kt + 1) * P, :])
                XK.append(xt)
            WQ = persist.tile([P, NKT, DQ], BF, tag="WQ")
            nc.sync.dma_start(out=WQ, in_=wq[:, :].rearrange("p (kt m) -> p kt m", m=DQ))
            XQA = persist.tile([P, NKT, S], BF, tag="xqa")
            nc.sync.dma_start(
                out=XQA[:, :, 0:QC],
                in_=xqT[:, 0:QC].rearrange("(kt p) q -> p kt q", p=P),
            )
            nc.sync.dma_start(out=BK0, in_=bk[0:P, :])
            nc.sync.dma_start(out=BK1, in_=bk[P:DQ, :])
            nc.sync.dma_start(out=BQ0, in_=bq[0:P, :])
            nc.sync.dma_start(out=BQ1, in_=bq[P:DQ, :])
            nc.sync.dma_start(out=BV, in_=bv[:, :].to_broadcast([P, DQ]))
            nc.sync.dma_start(out=VM, in_=vm[:, :])
            WV = persist.tile([P, NKT, DQ], BF, tag="WV")
            nc.sync.dma_start(out=WV, in_=wv[:, :].rearrange("p (kt m) -> p kt m", m=DQ))
            XVA = persist.tile([P, NKT, KP], BF, tag="xva")
            nc.sync.dma_start(
                out=XVA, in_=xvT[:, :].rearrange("(kt p) k -> p kt k", p=P)
            )
            nc.sync.dma_start(
                out=XQA[:, :, QC:S],
                in_=xqT[:, QC:S].rearrange("(kt p) q -> p kt q", p=P),
            )
            WO0 = persist.tile([P, DM], BF, tag="WO0")   # wo rows 0:128 (h0,h1)
            WO2 = persist.tile([DH, DM], BF, tag="WO2")  # wo rows 128:192 (h2)
            WO2x = persist.tile([P, DM], BF, tag="WO2x")  # dup on rows 64:128
            nc.sync.dma_start(out=WO0, in_=wo[0:P, :])
            nc.sync.dma_start(out=WO2, in_=wo[P:DQ, :])
            nc.sync.dma_start(out=WO2x[DH:P, :], in_=wo[P:DQ, :])

            # ---- persistent activations ----
            QT0 = persist.tile([P, S], BF, tag="QT0")    # heads 0,1
            QT1 = persist.tile([DH, S], BF, tag="QT1")   # head 2
            KT0 = persist.tile([P, KP], BF, tag="KT0")
            KT1 = persist.tile([DH, KP], BF, tag="KT1")  # head 2
            # V blocks of 128 cols per head (see module docstring)
            VP = persist.tile([P, T, HPG * P], BF, tag="VP")
            CTX01 = persist.tile([P, S], BF, tag="CTX01")  # h0 rows 0:64, h1 64:128
            CTX2 = persist.tile([DH, S], BF, tag="CTX2")   # h2
            CTX2x = persist.tile([P, S], BF, tag="CTX2x")  # h2 dup on rows 64:128

            ONES = persist.tile([P, HPG * DH], BF, tag="ONES")
            nc.vector.memset(ONES, 1.0)
            # pre-load the ACT exp table set during the idle projection phase
            WRM = persist.tile([1, 2], F32, tag="WRM")
            nc.vector.memset(WRM, 0.0)
            nc.scalar.activation(WRM[:, 1:2], WRM[:, 0:1], AFT.Exp)

            # ---- K projection (per key chunk; later chunks threaded into
            # the early attention iterations, DVE eviction) ----
            def kproj_chunk(kc, evict_on_act):
                c0, cw = KCH[kc]
                for m, (bias, mw) in enumerate([(BK0, P), (BK1, DH)]):
                    ps = ps_w.tile([P, 512], F32, tag="psw", name=f"kps{c0}_{m}")
                    for kt in range(NKT):
                        nc.tensor.matmul(
                            ps[0:mw, 0:cw],
                            lhsT=WK[:, kt, m * P:m * P + mw],
                            rhs=XK[kt][:, c0:c0 + cw],
                            start=(kt == 0),
                            stop=(kt == NKT - 1),
                        )
                    dst = KT0 if m == 0 else KT1
                    if evict_on_act:
                        nc.scalar.activation(
                            dst[0:mw, c0:c0 + cw], ps[0:mw, 0:cw],
                            AFT.Identity, bias=bias,
                        )
                    else:
                        nc.vector.tensor_scalar_add(
                            dst[0:mw, c0:c0 + cw], ps[0:mw, 0:cw], bias
                        )

            kproj_chunk(0, True)

            _qps = {}

            def qproj_part(c, m, half, evict_on_act=False):
                c0 = c * QC
                bias, mw = (BQ0, P) if m == 0 else (BQ1, DH)
                if half == 0:
                    _qps[(c, m)] = ps_w.tile(
                        [P, 512], F32, tag="psw", name=f"qps{c}_{m}"
                    )
                ps = _qps[(c, m)]
                kts = range(0, NKT // 2) if half == 0 else range(NKT // 2, NKT)
                for kt in kts:
                    nc.tensor.matmul(
                        ps[0:mw, :],
                        lhsT=WQ[:, kt, m * P:m * P + mw],
                        rhs=XQA[:, kt, c0:c0 + QC],
                        start=(kt == 0),
                        stop=(kt == NKT - 1),
                    )
                if half == 0:
                    return
                dst = QT0 if m == 0 else QT1
                if evict_on_act:
                    nc.scalar.activation(
                        dst[0:mw, c0:c0 + QC], ps[0:mw, :], AFT.Identity, bias=bias
                    )
                else:
                    nc.vector.tensor_scalar_add(
                        dst[0:mw, c0:c0 + QC], ps[0:mw, :], bias
                    )
                del _qps[(c, m)]

            def qproj_half(c, m, evict_on_act=False):
                qproj_part(c, m, 0, evict_on_act)
                qproj_part(c, m, 1, evict_on_act)

            def vproj(t):
                ps = ps_w.tile([P, 512], F32, tag="psw", name=f"vps{t}")
                for kt in range(NKT):
                    nc.tensor.matmul(
                        ps[:, 0:DQ],
                        lhsT=XVA[:, kt, t * P:(t + 1) * P],
                        rhs=WV[:, kt, :],
                        start=(kt == 0),
                        stop=(kt == NKT - 1),
                    )
                # all blocks [valid-ones 0:64 | V 64:128]: PV rows 0:64 = den
                # (partition base 0 for the fused psum reciprocal), 64:128 ctx
                vview = VP[:, t, :].rearrange("p (h c) -> p h c", c=P)
                nc.vector.tensor_add(
                    vview[:, :, DH:P],
                    ps[:, 0:DQ].rearrange("p (h d) -> p h d", d=DH),
                    BV[:, :].rearrange("p (h d) -> p h d", d=DH),
                )
                nc.vector.tensor_scalar_mul(
                    vview[:, :, DH:P], vview[:, :, DH:P], VM[:, t:t + 1]
                )
                nc.vector.tensor_scalar_mul(
                    vview[:, :, 0:DH],
                    ONES[:, :].rearrange("p (h d) -> p h d", d=DH),
                    VM[:, t:t + 1],
                )

            qproj_half(0, 0, evict_on_act=True)
            qproj_half(0, 1, evict_on_act=True)

            # ---- attention, software-pipelined emission ----
            # sp split into two PSUM tiles so the WAR of scores(i+1) on
            # exp(i) releases per-section: scores01 only waits exp01.
            sp01 = ps_sp.tile([P, 1024], F32, tag="sp01")  # [h0 512 | h1 512]
            sp2 = ps_sp.tile([P, 512], F32, tag="sp2")     # h2
            ctx = ps_ctx.tile([P, 1536], F32, tag="ctx")   # [h0 | h1 | h2]
            SCL = 1.0 / math.sqrt(DH)

            def scores01(c, t):
                c0 = c * QC
                tsl = slice(t * P, (t + 1) * P)
                nc.tensor.matmul(
                    sp01[:, 0:512],
                    lhsT=KT0[0:DH, tsl], rhs=QT0[0:DH, c0:c0 + QC],
                    start=True, stop=True,
                )
                nc.tensor.matmul(
                    sp01[:, 512:1024],
                    lhsT=KT0[DH:P, tsl], rhs=QT0[DH:P, c0:c0 + QC],
                    start=True, stop=True,
                )

            def scoresh2(c, t):
                c0 = c * QC
                tsl = slice(t * P, (t + 1) * P)
                nc.tensor.matmul(
                    sp2[:, 0:512],
                    lhsT=KT1[0:DH, tsl], rhs=QT1[0:DH, c0:c0 + QC],
                    start=True, stop=True,
                )

            def oproj_one(c, mo):
                # outT[mo*128:(mo+1)*128, chunk c] = WO0[:,mo].T@CTX01 + WO2[:,mo].T@CTX2
                c0 = c * QC
                po = ps_w.tile([P, 512], F32, tag="psw", name=f"po{c}_{mo}")
                osl = slice(mo * P, (mo + 1) * P)
                nc.tensor.matmul(
                    po, lhsT=WO0[:, osl], rhs=CTX01[:, c0:c0 + QC],
                    start=True, stop=False,
                )
                nc.tensor.matmul(
                    po, lhsT=WO2[:, osl], rhs=CTX2[0:DH, c0:c0 + QC],
                    start=False, stop=True,
                )
                po_sb = osb.tile([P, 512], BF, tag="posb", name=f"posb{c}_{mo}")
                nc.vector.tensor_copy(po_sb, po)
                nc.sync.dma_start(out=out[osl, c0:c0 + QC], in_=po_sb)

            def oproj_pair(c, mo):
                # tiles mo, mo+1: the two 64-contraction WO2 matmuls run in
                # disjoint PE row groups into different PSUM banks
                c0 = c * QC
                po_a = ps_w.tile([P, 512], F32, tag="psw", name=f"pa{c}_{mo}")
                po_b = ps_w.tile([P, 512], F32, tag="psw", name=f"pb{c}_{mo}")
                sa = slice(mo * P, (mo + 1) * P)
                sb = slice((mo + 1) * P, (mo + 2) * P)
                nc.tensor.matmul(
                    po_a, lhsT=WO0[:, sa], rhs=CTX01[:, c0:c0 + QC],
                    start=True, stop=False,
                )
                nc.tensor.matmul(
                    po_b, lhsT=WO0[:, sb], rhs=CTX01[:, c0:c0 + QC],
                    start=True, stop=False,
                )
                nc.tensor.matmul(
                    po_a, lhsT=WO2[:, sa], rhs=CTX2[0:DH, c0:c0 + QC],
                    start=False, stop=True,
                )
                nc.tensor.matmul(
                    po_b, lhsT=WO2x[DH:P, sb], rhs=CTX2x[DH:P, c0:c0 + QC],
                    start=False, stop=True,
                )
                for po_x, sx in ((po_a, sa), (po_b, sb)):
                    po_sb = osb.tile(
                        [P, 512], BF, tag="posb", name=f"ps{c}_{sx.start}"
                    )
                    nc.vector.tensor_copy(po_sb, po_x)
                    nc.sync.dma_start(out=out[sx, c0:c0 + QC], in_=po_sb)

            def pv(c, t):
                e01, e2 = es_hist[c * T + t]
                for h in range(HPG):
                    rhs = e01[:, h * 512:(h + 1) * 512] if h < 2 else e2[:, 0:512]
                    nc.tensor.matmul(
                        ctx[:, h * 512:h * 512 + 512],
                        lhsT=VP[:, t, h * P:(h + 1) * P],
                        rhs=rhs,
                        start=(t == 0), stop=(t == T - 1),
                    )

            def norm(c):
                # den rows 0:64 of every ctx section (partition base 0):
                # one fused reciprocal psum->sbuf, then per-head multiplies.
                c0 = c * QC
                rc = rcpool.tile([DH, 1536], F32, tag="rc", name=f"rc{c}")
                for h, (dst, dr) in enumerate([
                    (CTX01, slice(0, DH)),
                    (CTX01, slice(DH, P)),
                    (CTX2, slice(0, DH)),
                ]):
                    cs = slice(h * 512, (h + 1) * 512)
                    nc.vector.reciprocal_approx_fast(rc[:, cs], ctx[0:DH, cs])
                    nc.vector.tensor_mul(
                        dst[dr, c0:c0 + QC], ctx[DH:P, cs], rc[:, cs]
                    )
                nc.vector.tensor_mul(
                    CTX2x[DH:P, c0:c0 + QC], ctx[DH:P, 1024:1536],
                    rc[:, 1024:1536],
                )

            # Flat pipeline over i = c*T + t. Emission order per iteration:
            # exp01(i), exp2(i) [ACT] -> PV(i-LAG) + norm [PE/DVE] ->
            # threaded projections [PE] -> scores(i+1) [PE, LAST]. The PE
            # queue holds pv+projection fill work ahead of scores(i+1), so
            # the PE stays busy during exp(i); scores01(i+1) only waits on
            # exp01(i) (separate sp01/sp2 tiles), giving a steady period of
            # ~ACT-busy (1.8us) instead of the full exp+scores serial chain.
            NI = NQC * T
            LAG = 1
            es_hist = {}
            scores01(0, 0)
            scoresh2(0, 0)
            for i in range(NI + LAG):
                c, t = divmod(min(i, NI - 1), T)
                if i < NI:
                    e01 = espool.tile(
                        [P, 1024], BF, tag="es01", name=f"es01_{c}_{t}"
                    )
                    e2 = espool.tile(
                        [P, 512], BF, tag="es2", name=f"es2_{c}_{t}"
                    )
                    es_hist[c * T + t] = (e01, e2)
                    nc.scalar.activation(
                        e01, sp01, AFT.Exp, bias=0.0, scale=SCL
                    )
                    nc.scalar.activation(
                        e2, sp2, AFT.Exp, bias=0.0, scale=SCL
                    )
                if i >= LAG:
                    pc, pt = divmod(i - LAG, T)
                    # first PV of a chunk trails one extra iteration so the
                    # previous chunk's norm clears the ctx banks in time
                    if pt == 0 and T >= 3:
                        pass
                    elif pt == 1 and T >= 3:
                        pv(pc, 0)
                        es_hist.pop(pc * T)
                        pv(pc, 1)
                        es_hist.pop(pc * T + 1)
                    else:
                        pv(pc, pt)
                        es_hist.pop(pc * T + pt)
                    if pt == T - 1:
                        norm(pc)
                # threaded non-attention work, keyed by the exp index (c, t)
                if c == 0 and i < NI and t + 1 < len(KCH):
                    kproj_chunk(t + 1, False)
                if c == 0 and i < NI:
                    if t == 0:
                        vproj(0)
                        if T > 1:
                            vproj(1)
                    elif t + 1 < T:
                        vproj(t + 1)
                if c > 0 and i < NI and T >= 7 and t in (1, 3, 5):
                    oproj_pair(c - 1, t - 1)
                if c > 0 and i < NI and T < 7 and 1 <= t <= min(DM // P, T - 1):
                    oproj_one(c - 1, t - 1)
                if c + 1 < NQC and T >= 3 and i < NI:
                    if t == T - 3:
                        qproj_half(c + 1, 0)
                    elif t == T - 2:
                        qproj_half(c + 1, 1)
                if c + 1 < NQC and T < 3 and i < NI and t == T - 1:
                    qproj_half(c + 1, 0)
                    qproj_half(c + 1, 1)
                # leftover O-proj tiles when T is too small to thread them all
                if c > 0 and t == T - 1 and i < NI and T < 7:
                    for mo in range(min(DM // P, T - 1), DM // P):
                        oproj_one(c - 1, mo)
                # next scores LAST so all fill work is ahead in the PE queue
                if i < NI:
                    nxt_ok = i + 1 < NI and ((i + 1) % T != 0 or T >= 3)
                    if nxt_ok:
                        scores01(*divmod(i + 1, T))
                        scoresh2(*divmod(i + 1, T))
                    elif i + 1 < NI:
                        scores01(i // T + 1, 0)
                        scoresh2(i // T + 1, 0)
            if T >= 7:
                for mo in (0, 2, 4):
                    oproj_pair(NQC - 1, mo)
            else:
                for mo in range(DM // P):
                    oproj_one(NQC - 1, mo)
    nc.compile()
    return nc


def _get_prog(KP):
    if KP not in _prog_cache:
        _prog_cache[KP] = _build_nc(KP)
    return _prog_cache[KP]


def _rearrange_w(Wslice, BF):
    # [768, 192] -> [128, 6*192] (p-major kt blocks), contiguous for DMA
    return np.ascontiguousarray(
        Wslice.reshape(DM // P, P, DQ).transpose(1, 0, 2).reshape(P, -1)
    ).astype(BF)


def _run(inputs, trace=False):
    import ml_dtypes
    from concourse.bass_utils import run_bass_kernel_spmd

    BF = ml_dtypes.bfloat16

    query = np.asarray(inputs["query"], dtype=np.float32)
    key = np.asarray(inputs["key"], dtype=np.float32)
    value = np.asarray(inputs["value"], dtype=np.float32)
    mask = np.asarray(inputs["mask"])
    Wq = np.asarray(inputs["Wq"], dtype=np.float32)
    bq = np.asarray(inputs["bq"], dtype=np.float32)
    Wk = np.asarray(inputs["Wk"], dtype=np.float32)
    bk = np.asarray(inputs["bk"], dtype=np.float32)
    Wv = np.asarray(inputs["Wv"], dtype=np.float32)
    bv = np.asarray(inputs["bv"], dtype=np.float32)
    Wo = np.asarray(inputs["Wo"], dtype=np.float32)
    bo = np.asarray(inputs["bo"], dtype=np.float32)

    idx = [np.nonzero(mask[b, 0, 0] != 0)[0] for b in range(B)]
    keff = [len(i) for i in idx]
    KP = max(P, ((max(keff) + P - 1) // P) * P)
    T = KP // P

    nc = _get_prog(KP)

    per_batch = {}
    for b in range(B):
        xqT = np.ascontiguousarray(query[b].T).astype(BF)
        xkT = np.zeros((DM, KP), dtype=BF)
        xkT[:, :keff[b]] = key[b][idx[b]].T.astype(BF)
        xvT = np.zeros((DM, KP), dtype=BF)
        xvT[:, :keff[b]] = value[b][idx[b]].T.astype(BF)
        vmf = np.zeros((KP,), dtype=np.float32)
        vmf[:keff[b]] = 1.0
        vm2 = np.ascontiguousarray(vmf.reshape(T, P).T)  # [128, T]
        per_batch[b] = (xqT, xkT, xvT, vm2)

    in_maps = []
    for core in range(NCORES):
        b, g = core // G, core % G
        xqT, xkT, xvT, vm2 = per_batch[b]
        sl = slice(g * DQ, (g + 1) * DQ)
        in_maps.append({
            "xqT": xqT,
            "xkT": xkT,
            "xvT": xvT,
            "wq": _rearrange_w(Wq[:, sl], BF),
            "wk": _rearrange_w(Wk[:, sl], BF),
            "wv": _rearrange_w(Wv[:, sl], BF),
            "wo": np.ascontiguousarray(Wo[sl, :]).astype(BF),
            "bq": np.ascontiguousarray(bq[sl].reshape(DQ, 1)),
            "bk": np.ascontiguousarray(bk[sl].reshape(DQ, 1)),
            "bv": np.ascontiguousarray(bv[sl].reshape(1, DQ)),
            "vm": vm2,
        })

    res = run_bass_kernel_spmd(nc, in_maps, list(range(NCORES)), trace=trace)

    outp = np.zeros((B, S, DM), dtype=np.float32)
    for core in range(NCORES):
        outp[core // G] += np.asarray(res.results[core]["out"], dtype=np.float32).T
    outp += bo.reshape(1, 1, DM)
    return outp, res


def kernel(**inputs) -> np.ndarray:
    out, _ = _run(inputs, trace=False)
    return out


if __name__ == "__main__":
    nc = _build_nc(1152)
    print("build OK")



# revision 5
# speedup vs baseline: 1.1510x; 1.0371x over previous
"""Multi-head attention (B=2, S=2048, d_model=768, H=12) on 8 TRN2 NeuronCores.

Sharding: 2-way data parallel over batch x 4-way tensor parallel over heads
(3 heads / 192-wide d_model slice per core). Host compacts masked keys away
(gather of unmasked key/value rows), pads to a 128 multiple, zero-fills pad
keys; softmax needs no mask handling on device (pad keys get V=0 and 0s in
the denominator ones-block). Host pre-arranges every input into the exact
SBUF layout (chunk-major, partition-contiguous) so each tensor loads as one
or two DMAs of 128 large packets; the output is likewise stored as one wide
DMA per query chunk and re-assembled on host.

Per core, a software-pipelined flat loop over (chunk c of 512 q, key tile t)
with emission order: exp01/exp2 (ACT) -> PV(i-lag) + norm (PE/DVE) ->
threaded projections (PE) -> scores(i+1) LAST. The PE queue therefore holds
pv+projection fill work ahead of scores(i+1), which is the only instruction
that has to wait for exp(i) (WAR on the scores PSUM tiles). sp is split into
sp01 (2 banks, heads 0/1, row-group-paired matmuls) and sp2 (1 bank, head 2)
so scores01(i+1) only waits on exp01(i). Steady-state period ~= the ACT busy
time per iteration.

V blocks of 128 cols/head, all [valid-ones 64 | V 64]: PV lands a
64-row-replicated denominator on partitions 0:64 and ctx on 64:128 of one
3-bank ctx PSUM tile (denominator costs no PE time - matmul cost scales
with N only). Normalization per head section: one wide fast reciprocal
straight from PSUM (partition base 0 only) -> psum*recip multiplies to bf16.
The first PV of each chunk trails two extra iterations so the previous
chunk's norm clears the ctx banks before the in-order PE reaches the
start=True PV.

Output projection runs transposed (outT[dm,q] = Wo_g^T @ ctx), in
adjacent-tile pairs so the two 64-contraction WO2 matmuls run in disjoint
PE row groups into different PSUM banks; bf16 results collect in one wide
SBUF tile per chunk and fly out as a single DMA. A burst of warm-up matmuls
on scratch data runs during the initial DMA wait so the PE HAM clock-gate
releases (1.2 -> 2.4 GHz) before the first real matmul arrives.
"""

import math

import numpy as np

B = 2
S = 2048
DM = 768
H = 12
DH = 64
G = 4              # head-group (tensor-parallel) degree
HPG = H // G       # heads per core
DQ = HPG * DH      # 192 d_model slice per core
NCORES = 8
P = 128
QC = 512           # query chunk
NQC = S // QC
NKT = DM // P      # 6 contraction tiles for projections
NMO = DM // P      # 6 output-projection tiles

_prog_cache = {}


def _build_nc(KP):
    import concourse.mybir as mybir
    import concourse.tile as tile
    from concourse import bacc

    F32 = mybir.dt.float32
    BF = mybir.dt.bfloat16
    AFT = mybir.ActivationFunctionType

    T = KP // P            # key tiles
    KCH = [(o, min(512, KP - o)) for o in range(0, KP, 512)]
    MW = 4 + T + DQ        # misc tensor cols: biases | vm | bv

    nc = bacc.Bacc(None, target_bir_lowering=False)
    # all inputs host-pre-arranged to [128, *] partition-contiguous layouts
    xq = nc.declare_dram_parameter("xq", [P, NQC * NKT * QC], BF, isOutput=False)
    xk = nc.declare_dram_parameter("xk", [P, NKT * KP], BF, isOutput=False)
    xv = nc.declare_dram_parameter("xv", [P, NKT * KP], BF, isOutput=False)
    wq = nc.declare_dram_parameter("wq", [P, NKT * DQ], BF, isOutput=False)
    wk = nc.declare_dram_parameter("wk", [P, NKT * DQ], BF, isOutput=False)
    wv = nc.declare_dram_parameter("wv", [P, NKT * DQ], BF, isOutput=False)
    wop = nc.declare_dram_parameter("wop", [P, 2 * DM], BF, isOutput=False)
    msc = nc.declare_dram_parameter("msc", [P, MW], F32, isOutput=False)
    out = nc.declare_dram_parameter("out", [P, NQC * NMO * QC], BF, isOutput=True)

    with tile.TileContext(nc) as tc:
        with (
            tc.tile_pool(name="persist", bufs=1) as persist,
            tc.tile_pool(name="es", bufs=5) as espool,
            tc.tile_pool(name="rc", bufs=2) as rcpool,
            tc.tile_pool(name="osb", bufs=2) as osb,
            tc.tile_pool(name="ps_sp", bufs=1, space="PSUM") as ps_sp,
            tc.tile_pool(name="ps_ctx", bufs=1, space="PSUM") as ps_ctx,
            tc.tile_pool(name="ps_w", bufs=2, space="PSUM") as ps_w,
        ):
            # ---- warm-up scratch + exp-table preload ----
            WUP = persist.tile([P, 512], BF, tag="WUP")
            nc.gpsimd.memset(WUP, 0.0)
            WRM = persist.tile([1, 2], F32, tag="WRM")
            nc.vector.memset(WRM, 0.0)
            nc.scalar.activation(WRM[:, 1:2], WRM[:, 0:1], AFT.Exp)
            for w in range(10):
                wps = ps_w.tile([P, 512], F32, tag="psw", name=f"warm{w}")
                nc.tensor.matmul(
                    wps, lhsT=WUP[:, 0:P], rhs=WUP, start=True, stop=True
                )

            # ---- constants / weights / activations (K path first) ----
            WK = persist.tile([P, NKT, DQ], BF, tag="WK")
            nc.sync.dma_start(out=WK, in_=wk[:, :].rearrange("p (kt m) -> p kt m", m=DQ))
            XKA = persist.tile([P, NKT * KP], BF, tag="XKA")
            c00, cw0 = KCH[0]
            nc.sync.dma_start(out=XKA[:, 0:NKT * cw0], in_=xk[:, 0:NKT * cw0])
            MISC = persist.tile([P, MW], F32, tag="MISC")
            nc.sync.dma_start(out=MISC, in_=msc[:, :])
            WQ = persist.tile([P, NKT, DQ], BF, tag="WQ")
            nc.sync.dma_start(out=WQ, in_=wq[:, :].rearrange("p (kt m) -> p kt m", m=DQ))
            XQA = persist.tile([P, NQC * NKT * QC], BF, tag="XQA")
            nc.sync.dma_start(
                out=XQA[:, 0:NKT * QC], in_=xq[:, 0:NKT * QC]
            )
            if KP > cw0:
                nc.sync.dma_start(
                    out=XKA[:, NKT * cw0:], in_=xk[:, NKT * cw0:]
                )
            WV = persist.tile([P, NKT, DQ], BF, tag="WV")
            nc.sync.dma_start(out=WV, in_=wv[:, :].rearrange("p (kt m) -> p kt m", m=DQ))
            XVA = persist.tile([P, NKT * KP], BF, tag="XVA")
            nc.sync.dma_start(out=XVA[:, 0:NKT * cw0], in_=xv[:, 0:NKT * cw0])
            nc.sync.dma_start(
                out=XQA[:, NKT * QC:], in_=xq[:, NKT * QC:]
            )
            if KP > cw0:
                nc.sync.dma_start(
                    out=XVA[:, NKT * cw0:], in_=xv[:, NKT * cw0:]
                )
            WOP = persist.tile([P, 2 * DM], BF, tag="WOP")
            nc.sync.dma_start(out=WOP, in_=wop[:, :])

            # misc views
            BQ0 = MISC[:, 0:1]
            BQ1 = MISC[0:DH, 1:2]
            BK0 = MISC[:, 2:3]
            BK1 = MISC[0:DH, 3:4]
            VM = MISC[:, 4:4 + T]
            BV = MISC[:, 4 + T:4 + T + DQ]
            WO0 = WOP[:, 0:DM]          # wo rows 0:128 (h0,h1)
            WO2 = WOP[0:DH, DM:2 * DM]  # wo rows 128:192 (h2)
            WO2x = WOP[:, DM:2 * DM]    # same, duplicated on rows 64:128

            # ---- persistent activations ----
            QT0 = persist.tile([P, S], BF, tag="QT0")    # heads 0,1
            QT1 = persist.tile([DH, S], BF, tag="QT1")   # head 2
            KT0 = persist.tile([P, KP], BF, tag="KT0")
            KT1 = persist.tile([DH, KP], BF, tag="KT1")  # head 2
            # V blocks of 128 cols per head (see module docstring)
            VP = persist.tile([P, T, HPG * P], BF, tag="VP")
            CTX01 = persist.tile([P, S], BF, tag="CTX01")  # h0 rows 0:64, h1 64:128
            CTX2 = persist.tile([DH, S], BF, tag="CTX2")   # h2
            CTX2x = persist.tile([P, S], BF, tag="CTX2x")  # h2 dup on rows 64:128

            ONES = persist.tile([P, HPG * DH], BF, tag="ONES")
            nc.vector.memset(ONES, 1.0)

            # ---- K projection (per key chunk; later chunks threaded into
            # the early attention iterations, DVE eviction) ----
            def kproj_chunk(kc, evict_on_act):
                c0, cw = KCH[kc]
                for m, (bias, mw) in enumerate([(BK0, P), (BK1, DH)]):
                    ps = ps_w.tile([P, 512], F32, tag="psw", name=f"kps{c0}_{m}")
                    for kt in range(NKT):
                        nc.tensor.matmul(
                            ps[0:mw, 0:cw],
                            lhsT=WK[:, kt, m * P:m * P + mw],
                            rhs=XKA[:, c0 * NKT + kt * cw:c0 * NKT + (kt + 1) * cw],
                            start=(kt == 0),
                            stop=(kt == NKT - 1),
                        )
                    dst = KT0 if m == 0 else KT1
                    if evict_on_act:
                        nc.scalar.activation(
                            dst[0:mw, c0:c0 + cw], ps[0:mw, 0:cw],
                            AFT.Identity, bias=bias,
                        )
                    else:
                        nc.vector.tensor_scalar_add(
                            dst[0:mw, c0:c0 + cw], ps[0:mw, 0:cw], bias
                        )

            kproj_chunk(0, True)

            _qps = {}

            def qproj_part(c, m, half, evict_on_act=False):
                c0 = c * QC
                bias, mw = (BQ0, P) if m == 0 else (BQ1, DH)
                if half == 0:
                    _qps[(c, m)] = ps_w.tile(
                        [P, 512], F32, tag="psw", name=f"qps{c}_{m}"
                    )
                ps = _qps[(c, m)]
                kts = range(0, NKT // 2) if half == 0 else range(NKT // 2, NKT)
                for kt in kts:
                    nc.tensor.matmul(
                        ps[0:mw, :],
                        lhsT=WQ[:, kt, m * P:m * P + mw],
                        rhs=XQA[:, (c * NKT + kt) * QC:(c * NKT + kt + 1) * QC],
                        start=(kt == 0),
                        stop=(kt == NKT - 1),
                    )
                if half == 0:
                    return
                dst = QT0 if m == 0 else QT1
                if evict_on_act:
                    nc.scalar.activation(
                        dst[0:mw, c0:c0 + QC], ps[0:mw, :], AFT.Identity, bias=bias
                    )
                else:
                    nc.vector.tensor_scalar_add(
                        dst[0:mw, c0:c0 + QC], ps[0:mw, :], bias
                    )
                del _qps[(c, m)]

            def qproj_half(c, m, evict_on_act=False):
                qproj_part(c, m, 0, evict_on_act)
                qproj_part(c, m, 1, evict_on_act)

            def vproj(t):
                kc = min(t * P // 512, len(KCH) - 1)
                c0, cw = KCH[kc]
                ps = ps_w.tile([P, 512], F32, tag="psw", name=f"vps{t}")
                for kt in range(NKT):
                    off = c0 * NKT + kt * cw + (t * P - c0)
                    nc.tensor.matmul(
                        ps[:, 0:DQ],
                        lhsT=XVA[:, off:off + P],
                        rhs=WV[:, kt, :],
                        start=(kt == 0),
                        stop=(kt == NKT - 1),
                    )
                # all blocks [valid-ones 0:64 | V 64:128]: PV rows 0:64 = den
                # (partition base 0 for the fused psum reciprocal), 64:128 ctx
                vview = VP[:, t, :].rearrange("p (h c) -> p h c", c=P)
                nc.vector.tensor_add(
                    vview[:, :, DH:P],
                    ps[:, 0:DQ].rearrange("p (h d) -> p h d", d=DH),
                    BV[:, :].rearrange("p (h d) -> p h d", d=DH),
                )
                nc.vector.tensor_scalar_mul(
                    vview[:, :, DH:P], vview[:, :, DH:P], VM[:, t:t + 1]
                )
                nc.vector.tensor_scalar_mul(
                    vview[:, :, 0:DH],
                    ONES[:, :].rearrange("p (h d) -> p h d", d=DH),
                    VM[:, t:t + 1],
                )

            qproj_half(0, 0, evict_on_act=True)
            qproj_half(0, 1, evict_on_act=True)

            # ---- attention, software-pipelined emission ----
            # sp split into two PSUM tiles so the WAR of scores(i+1) on
            # exp(i) releases per-section: scores01 only waits exp01.
            sp01 = ps_sp.tile([P, 1024], F32, tag="sp01")  # [h0 512 | h1 512]
            sp2 = ps_sp.tile([P, 512], F32, tag="sp2")     # h2
            ctx = ps_ctx.tile([P, 1536], F32, tag="ctx")   # [h0 | h1 | h2]
            SCL = 1.0 / math.sqrt(DH)

            def scores01(c, t):
                c0 = c * QC
                tsl = slice(t * P, (t + 1) * P)
                nc.tensor.matmul(
                    sp01[:, 0:512],
                    lhsT=KT0[0:DH, tsl], rhs=QT0[0:DH, c0:c0 + QC],
                    start=True, stop=True,
                )
                nc.tensor.matmul(
                    sp01[:, 512:1024],
                    lhsT=KT0[DH:P, tsl], rhs=QT0[DH:P, c0:c0 + QC],
                    start=True, stop=True,
                )

            def scoresh2(c, t):
                c0 = c * QC
                tsl = slice(t * P, (t + 1) * P)
                nc.tensor.matmul(
                    sp2[:, 0:512],
                    lhsT=KT1[0:DH, tsl], rhs=QT1[0:DH, c0:c0 + QC],
                    start=True, stop=True,
                )

            _osb = {}

            def osb_for(c):
                if c not in _osb:
                    _osb[c] = osb.tile(
                        [P, NMO * QC], BF, tag="posb", name=f"osb{c}"
                    )
                return _osb[c]

            def oproj_flush(c):
                nc.sync.dma_start(
                    out=out[:, c * NMO * QC:(c + 1) * NMO * QC],
                    in_=_osb.pop(c),
                )

            def oproj_one(c, mo, evict_act=False):
                # outT[mo*128:(mo+1)*128, chunk c] = WO0[:,mo].T@CTX01 + WO2[:,mo].T@CTX2
                c0 = c * QC
                po = ps_w.tile([P, 512], F32, tag="psw", name=f"po{c}_{mo}")
                osl = slice(mo * DM, mo * DM + DM)
                nc.tensor.matmul(
                    po, lhsT=WO0[:, mo * P:(mo + 1) * P],
                    rhs=CTX01[:, c0:c0 + QC],
                    start=True, stop=False,
                )
                nc.tensor.matmul(
                    po, lhsT=WO2[:, mo * P:(mo + 1) * P],
                    rhs=CTX2[0:DH, c0:c0 + QC],
                    start=False, stop=True,
                )
                dst = osb_for(c)[:, mo * QC:(mo + 1) * QC]
                if evict_act:
                    nc.scalar.activation(dst, po, AFT.Identity, bias=0.0)
                else:
                    nc.vector.tensor_copy(dst, po)

            def oproj_pair(c, mo, evict_act=False):
                # tiles mo, mo+1: the two 64-contraction WO2 matmuls run in
                # disjoint PE row groups into different PSUM banks
                c0 = c * QC
                po_a = ps_w.tile([P, 512], F32, tag="psw", name=f"pa{c}_{mo}")
                po_b = ps_w.tile([P, 512], F32, tag="psw", name=f"pb{c}_{mo}")
                sa = slice(mo * P, (mo + 1) * P)
                sb = slice((mo + 1) * P, (mo + 2) * P)
                nc.tensor.matmul(
                    po_a, lhsT=WO0[:, sa], rhs=CTX01[:, c0:c0 + QC],
                    start=True, stop=False,
                )
                nc.tensor.matmul(
                    po_b, lhsT=WO0[:, sb], rhs=CTX01[:, c0:c0 + QC],
                    start=True, stop=False,
                )
                nc.tensor.matmul(
                    po_a, lhsT=WO2[:, sa], rhs=CTX2[0:DH, c0:c0 + QC],
                    start=False, stop=True,
                )
                nc.tensor.matmul(
                    po_b, lhsT=WO2x[DH:P, sb], rhs=CTX2x[DH:P, c0:c0 + QC],
                    start=False, stop=True,
                )
                ot = osb_for(c)
                for k, po_x in enumerate((po_a, po_b)):
                    dst = ot[:, (mo + k) * QC:(mo + k + 1) * QC]
                    if evict_act and k == 0:
                        nc.scalar.activation(dst, po_x, AFT.Identity, bias=0.0)
                    else:
                        nc.vector.tensor_copy(dst, po_x)

            def pv(c, t):
                e01, e2 = es_hist[c * T + t]
                for h in range(HPG):
                    rhs = e01[:, h * 512:(h + 1) * 512] if h < 2 else e2[:, 0:512]
                    nc.tensor.matmul(
                        ctx[:, h * 512:h * 512 + 512],
                        lhsT=VP[:, t, h * P:(h + 1) * P],
                        rhs=rhs,
                        start=(t == 0), stop=(t == T - 1),
                    )

            def norm(c):
                # den rows 0:64 of every ctx section (partition base 0):
                # one wide fused reciprocal psum->sbuf, then per-head
                # multiplies.
                c0 = c * QC
                rc = rcpool.tile([DH, 1536], F32, tag="rc", name=f"rc{c}")
                nc.vector.reciprocal_approx_fast(rc, ctx[0:DH, :])
                for h, (dst, dr) in enumerate([
                    (CTX01, slice(0, DH)),
                    (CTX01, slice(DH, P)),
                    (CTX2, slice(0, DH)),
                ]):
                    cs = slice(h * 512, (h + 1) * 512)
                    nc.vector.tensor_mul(
                        dst[dr, c0:c0 + QC], ctx[DH:P, cs], rc[:, cs]
                    )
                nc.vector.tensor_mul(
                    CTX2x[DH:P, c0:c0 + QC], ctx[DH:P, 1024:1536],
                    rc[:, 1024:1536],
                )

            # Flat pipeline over i = c*T + t. Emission order per iteration:
            # exp01(i), exp2(i) [ACT] -> PV(i-LAG) + norm [PE/DVE] ->
            # threaded projections [PE] -> scores(i+1) [PE, LAST]. The first
            # PV of a chunk trails two extra iterations so the previous
            # chunk's norm (DVE) clears the ctx banks before the in-order PE
            # reaches the start=True PV.
            NI = NQC * T
            LAG = 1
            es_hist = {}
            scores01(0, 0)
            scoresh2(0, 0)
            for i in range(NI + LAG):
                c, t = divmod(min(i, NI - 1), T)
                if i < NI:
                    e01 = espool.tile(
                        [P, 1024], BF, tag="es01", name=f"es01_{c}_{t}"
                    )
                    e2 = espool.tile(
                        [P, 512], BF, tag="es2", name=f"es2_{c}_{t}"
                    )
                    es_hist[c * T + t] = (e01, e2)
                    nc.scalar.activation(
                        e01, sp01, AFT.Exp, bias=0.0, scale=SCL
                    )
                    nc.scalar.activation(
                        e2, sp2, AFT.Exp, bias=0.0, scale=SCL
                    )
                if i >= LAG:
                    pc, pt = divmod(i - LAG, T)
                    if T >= 4:
                        if pt in (0, 1):
                            pass
                        elif pt == 2:
                            for pu in range(3):
                                pv(pc, pu)
                                es_hist.pop(pc * T + pu)
                        else:
                            pv(pc, pt)
                            es_hist.pop(pc * T + pt)
                    elif T >= 3:
                        if pt == 0:
                            pass
                        elif pt == 1:
                            pv(pc, 0)
                            es_hist.pop(pc * T)
                            pv(pc, 1)
                            es_hist.pop(pc * T + 1)
                        else:
                            pv(pc, pt)
                            es_hist.pop(pc * T + pt)
                    else:
                        pv(pc, pt)
                        es_hist.pop(pc * T + pt)
                    if pt == T - 1:
                        norm(pc)
                # threaded non-attention work, keyed by the exp index (c, t)
                if c == 0 and i < NI and t + 1 < len(KCH):
                    kproj_chunk(t + 1, False)
                if c == 0 and i < NI:
                    if t == 0:
                        vproj(0)
                        if T > 1:
                            vproj(1)
                    elif t + 1 < T:
                        vproj(t + 1)
                if c > 0 and i < NI and T >= 7 and t in (1, 3, 5):
                    oproj_pair(c - 1, t - 1)
                    if t == 5:
                        oproj_flush(c - 1)
                if c > 0 and i < NI and T < 7 and 1 <= t <= min(NMO, T - 1):
                    oproj_one(c - 1, t - 1)
                if c + 1 < NQC and T >= 3 and i < NI:
                    if t == T - 3:
                        qproj_half(c + 1, 0)
                    elif t == T - 2:
                        qproj_half(c + 1, 1)
                if c + 1 < NQC and T < 3 and i < NI and t == T - 1:
                    qproj_half(c + 1, 0)
                    qproj_half(c + 1, 1)
                # leftover O-proj tiles when T is too small to thread them all
                if c > 0 and t == T - 1 and i < NI and T < 7:
                    for mo in range(min(NMO, T - 1), NMO):
                        oproj_one(c - 1, mo)
                    oproj_flush(c - 1)
                # next scores LAST so all fill work is ahead in the PE queue
                if i < NI:
                    nxt_ok = i + 1 < NI and ((i + 1) % T != 0 or T >= 3)
                    if nxt_ok:
                        scores01(*divmod(i + 1, T))
                        scoresh2(*divmod(i + 1, T))
                    elif i + 1 < NI:
                        scores01(i // T + 1, 0)
                        scoresh2(i // T + 1, 0)
            # tail: last chunk's O-projection; evictions alternate ACT/DVE
            # (ACT is idle once the exps are done)
            if T >= 7:
                for mo in (0, 2, 4):
                    oproj_pair(NQC - 1, mo, evict_act=True)
            else:
                for mo in range(NMO):
                    oproj_one(NQC - 1, mo, evict_act=(mo % 2 == 0))
            oproj_flush(NQC - 1)
    nc.compile()
    return nc


def _get_prog(KP):
    if KP not in _prog_cache:
        _prog_cache[KP] = _build_nc(KP)
    return _prog_cache[KP]


def _rearrange_w(Wslice, BF):
    # [768, 192] -> [128, 6*192] (p-major kt blocks), contiguous for DMA
    return np.ascontiguousarray(
        Wslice.reshape(DM // P, P, DQ).transpose(1, 0, 2).reshape(P, -1)
    ).astype(BF)


def _chunk_major(xT, KCH):
    # [768, KP] -> [128, NKT*KP] with per-partition layout [kc][kt][cols]
    x3 = xT.reshape(NKT, P, xT.shape[1])
    return np.concatenate(
        [
            np.ascontiguousarray(
                x3[:, :, c0:c0 + cw].transpose(1, 0, 2)
            ).reshape(P, NKT * cw)
            for c0, cw in KCH
        ],
        axis=1,
    )


def _run(inputs, trace=False):
    import ml_dtypes
    from concourse.bass_utils import run_bass_kernel_spmd

    BF = ml_dtypes.bfloat16

    query = np.asarray(inputs["query"], dtype=np.float32)
    key = np.asarray(inputs["key"], dtype=np.float32)
    value = np.asarray(inputs["value"], dtype=np.float32)
    mask = np.asarray(inputs["mask"])
    Wq = np.asarray(inputs["Wq"], dtype=np.float32)
    bq = np.asarray(inputs["bq"], dtype=np.float32)
    Wk = np.asarray(inputs["Wk"], dtype=np.float32)
    bk = np.asarray(inputs["bk"], dtype=np.float32)
    Wv = np.asarray(inputs["Wv"], dtype=np.float32)
    bv = np.asarray(inputs["bv"], dtype=np.float32)
    Wo = np.asarray(inputs["Wo"], dtype=np.float32)
    bo = np.asarray(inputs["bo"], dtype=np.float32)

    idx = [np.nonzero(mask[b, 0, 0] != 0)[0] for b in range(B)]
    keff = [len(i) for i in idx]
    KP = max(P, ((max(keff) + P - 1) // P) * P)
    T = KP // P
    KCH = [(o, min(512, KP - o)) for o in range(0, KP, 512)]

    nc = _get_prog(KP)

    per_batch = {}
    for b in range(B):
        # q: [128, NQC*NKT*QC] chunk-major ([c][kt][q])
        xqT = query[b].T.astype(BF)  # [768, 2048]
        xq_p = np.ascontiguousarray(
            xqT.reshape(NKT, P, NQC, QC).transpose(1, 2, 0, 3)
        ).reshape(P, -1)
        xkT = np.zeros((DM, KP), dtype=BF)
        xkT[:, :keff[b]] = key[b][idx[b]].T.astype(BF)
        xvT = np.zeros((DM, KP), dtype=BF)
        xvT[:, :keff[b]] = value[b][idx[b]].T.astype(BF)
        xk_p = _chunk_major(xkT, KCH)
        xv_p = _chunk_major(xvT, KCH)
        vmf = np.zeros((KP,), dtype=np.float32)
        vmf[:keff[b]] = 1.0
        vm2 = np.ascontiguousarray(vmf.reshape(T, P).T)  # [128, T]
        per_batch[b] = (xq_p, xk_p, xv_p, vm2)

    in_maps = []
    for core in range(NCORES):
        b, g = core // G, core % G
        xq_p, xk_p, xv_p, vm2 = per_batch[b]
        sl = slice(g * DQ, (g + 1) * DQ)
        wo_g = Wo[sl, :].astype(BF)  # [192, 768]
        wo_pack = np.concatenate(
            [wo_g[0:P], np.concatenate([wo_g[P:DQ], wo_g[P:DQ]], axis=0)],
            axis=1,
        )  # [128, 1536]
        mw = 4 + T + DQ
        msc = np.zeros((P, mw), dtype=np.float32)
        msc[:, 0] = bq[sl][0:P]
        msc[0:DH, 1] = bq[sl][P:DQ]
        msc[:, 2] = bk[sl][0:P]
        msc[0:DH, 3] = bk[sl][P:DQ]
        msc[:, 4:4 + T] = vm2
        msc[:, 4 + T:] = bv[sl][None, :]
        in_maps.append({
            "xq": xq_p,
            "xk": xk_p,
            "xv": xv_p,
            "wq": _rearrange_w(Wq[:, sl], BF),
            "wk": _rearrange_w(Wk[:, sl], BF),
            "wv": _rearrange_w(Wv[:, sl], BF),
            "wop": np.ascontiguousarray(wo_pack),
            "msc": msc,
        })

    res = run_bass_kernel_spmd(nc, in_maps, list(range(NCORES)), trace=trace)

    outp = np.zeros((B, S, DM), dtype=np.float32)
    for core in range(NCORES):
        o = np.asarray(res.results[core]["out"], dtype=np.float32)
        outT = o.reshape(P, NQC, NMO, QC).transpose(2, 0, 1, 3).reshape(DM, S)
        outp[core // G] += outT.T
    outp += bo.reshape(1, 1, DM)
    return outp, res


def kernel(**inputs) -> np.ndarray:
    out, _ = _run(inputs, trace=False)
    return out


if __name__ == "__main__":
    nc = _build_nc(1152)
    print("build OK")
